# revision 21
# baseline (speedup 1.0000x reference)
"""Overlapping-windows (conv1d-identity unfold) kernel for Trainium2.

out[b*T + t, w*C + c] = x[b, t + w - CTX, c]  (zero-padded in t), i.e. each
output row is a contiguous 494-element window of the zero-padded, flattened
per-batch [T + 2*CTX, C] array starting at t*C.

Strategy:
  - Shard batch dim across 8 cores (8 batches/core).
  - Per core, stage the padded input in SBUF as 128 partitions =
    8 batches x 16 time-chunks; partition p = 16*b + j holds padded rows
    [j*K, j*K + K + 2*CTX) of batch b  (K = 125 rows, 3718 f32/partition).
    Since T*C == NCHUNK*K*C, the per-partition source offset is affine in
    p, so the whole halo load is ONE line-rate DMA over partitions 1..126
    (edge partitions clip at the tensor bounds and load separately); the
    cross-batch garbage this leaves in the 18-row halo strips at batch
    boundaries is then overwritten by zero-strip DMAs from a NEFF-embedded
    constant. The load is split into two column-waves so the first unfold
    pass (which only reads rows [0,43) of each chunk) starts early.
  - 5 pipelined passes: DVE + ACT copy-unfold 25 output rows per partition
    into a triple-buffered tile ys[128, 25*494] (per-partition strided
    overlapping reads from xs), then one outbound DMA per pass writes
    6.3 MB with 49 KB contiguous descriptors (both sides contiguous per
    partition, ~430 GB/s). Big descriptors amortize the per-descriptor
    DMA-engine overhead that limits a direct 1976 B-descriptor store to
    ~250 GB/s; triple buffering keeps the outbound queue busy back-to-back.
"""

import numpy as np

N_CTX = 9
C = 26
W = 2 * N_CTX + 1          # 19
ROWLEN = W * C             # 494
B, T = 64, 2000
N_CORES = 8
B_C = B // N_CORES         # 8 batches per core
NCHUNK = 16                # time-chunks per batch -> 8*16 = 128 partitions
K = T // NCHUNK            # 125 rows per chunk
PF = (K + 2 * N_CTX) * C   # 3718 f32 per partition (chunk + halo)
EDGE = (K + N_CTX) * C     # 3484 f32 (edge partitions, one-sided halo)
HALO = N_CTX * C           # 234 f32

NPASS = 5
NR = K // NPASS            # 25 output rows per partition per pass
VROWS = 11                 # rows unfolded by DVE per pass (ACT does the rest)
NBUF = 3                   # ys staging buffers
YF = NR * ROWLEN           # 12350 f32 per partition per staging buffer
W1 = (NR + 2 * N_CTX) * C  # 1118 f32: wave-1 columns (rows [0,43) per chunk)


def _build_nc():
    import concourse.bass as bass
    import concourse.mybir as mybir

    nc = bass.Bass(target_bir_lowering=False)
    x = nc.dram_tensor("x", [B_C, T, C], mybir.dt.float32, kind="ExternalInput")
    out = nc.dram_tensor(
        "out", [B_C * T, ROWLEN], mybir.dt.float32, kind="ExternalOutput"
    )

    with (
        nc.sbuf_tensor("xs", [128, PF], mybir.dt.float32) as xs,
        nc.sbuf_tensor("ys0", [128, YF], mybir.dt.float32) as ys0,
        nc.sbuf_tensor("ys1", [128, YF], mybir.dt.float32) as ys1,
        nc.sbuf_tensor("ys2", [128, YF], mybir.dt.float32) as ys2,
        nc.semaphore("in_sem") as in_sem,
        nc.semaphore("s_sem") as s_sem,
        nc.semaphore("t_sem") as t_sem,
        nc.semaphore("uv_sem") as uv_sem,
        nc.semaphore("ua_sem") as ua_sem,
        nc.semaphore("o_sem") as o_sem,
        nc.Block() as block,
    ):
        ys = [ys0, ys1, ys2]

        zeros = nc.inline_tensor(
            np.zeros(B_C * HALO, dtype=np.float32), name="zstrip"
        )

        @block.sync
        def _(sync):
            # wave 1: columns [0, W1) of partitions 1..126; src offset for
            # partition p is p*K*C - HALO (affine because T*C == NCHUNK*K*C)
            sync.dma_start(
                bass.AP(xs, PF, [[PF, 126], [1, W1]]),
                bass.AP(x, K * C - HALO, [[K * C, 126], [1, W1]]),
            ).then_inc(in_sem, 16)
            # wave 2: columns [W1, PF) of partitions 1..126
            sync.dma_start(
                bass.AP(xs, PF + W1, [[PF, 126], [1, PF - W1]]),
                bass.AP(x, K * C - HALO + W1, [[K * C, 126], [1, PF - W1]]),
            ).then_inc(in_sem, 16)
            # ---- outbound: one DMA per pass, 49 KB descriptors ----
            for m in range(NPASS):
                sync.wait_ge(uv_sem, m + 1)
                sync.wait_ge(ua_sem, m + 1)
                sync.dma_start(
                    bass.AP(
                        out,
                        m * NR * ROWLEN,
                        [[K * ROWLEN, 128], [1, YF]],
                    ),
                    bass.AP(ys[m % NBUF], 0, [[YF, 128], [1, YF]]),
                ).then_inc(o_sem, 16)
            sync.wait_ge(o_sem, 16 * NPASS)

        @block.scalar
        def _(scalar):
            # dummy 1-element copy to preload the ACT identity table during
            # the inbound phase (harmless: ys2 is fully rewritten by its
            # unfold pass before any outbound reads it)
            scalar.copy(
                bass.AP(ys2, 0, [[YF, 1], [1, 1]]),
                bass.AP(xs, 0, [[PF, 1], [1, 1]]),
            )
            # edge partitions: p=0 (batch 0 head, dst offset HALO) and
            # p=127 (batch 7 tail, dst offset 0), each 134 rows
            scalar.dma_start(
                bass.AP(xs, HALO, [[PF, 1], [1, EDGE]]),
                bass.AP(x, 0, [[EDGE, 1], [1, EDGE]]),
            ).then_inc(s_sem, 16)
            scalar.dma_start(
                bass.AP(xs, 127 * PF, [[PF, 1], [1, EDGE]]),
                bass.AP(x, 127 * K * C - HALO, [[EDGE, 1], [1, EDGE]]),
            ).then_inc(s_sem, 16)
            # head zero strips (chunk 0 of each batch, cols [0, HALO)) —
            # must follow wave 1, which writes cross-batch garbage there
            scalar.wait_ge(in_sem, 16)
            scalar.dma_start(
                bass.AP(xs, 0, [[NCHUNK * PF, B_C], [1, HALO]]),
                bass.AP(zeros, 0, [[HALO, B_C], [1, HALO]]),
            ).then_inc(s_sem, 16)

            # unfold rows [m*NR+VROWS, (m+1)*NR) of each partition
            for m in range(NPASS):
                scalar.wait_ge(in_sem, 16 if m == 0 else 32)
                scalar.wait_ge(s_sem, 48)
                if m == NPASS - 1:
                    scalar.wait_ge(t_sem, 16)  # only pass 4 reads tail strips
                if m >= NBUF:
                    scalar.wait_ge(o_sem, 16 * (m - NBUF + 1))
                scalar.copy(
                    bass.AP(
                        ys[m % NBUF],
                        VROWS * ROWLEN,
                        [[YF, 128], [ROWLEN, NR - VROWS], [1, ROWLEN]],
                    ),
                    bass.AP(
                        xs,
                        (m * NR + VROWS) * C,
                        [[PF, 128], [C, NR - VROWS], [1, ROWLEN]],
                    ),
                ).then_inc(ua_sem, 1)

        @block.gpsimd
        def _(gpsimd):
            # tail zero strips (chunk 15 of each batch, cols [EDGE, PF)):
            # wave 2 writes cross-batch garbage there, so wait for it; only
            # unfold pass 4 reads this region, so there's ample slack.
            gpsimd.wait_ge(in_sem, 32)
            gpsimd.dma_start(
                bass.AP(
                    xs,
                    (NCHUNK - 1) * PF + EDGE,
                    [[NCHUNK * PF, B_C], [1, HALO]],
                ),
                bass.AP(zeros, 0, [[HALO, B_C], [1, HALO]]),
            ).then_inc(t_sem, 16)

        @block.vector
        def _(vector):
            # unfold rows [m*NR, m*NR+VROWS) of each partition
            for m in range(NPASS):
                vector.wait_ge(in_sem, 16 if m == 0 else 32)
                vector.wait_ge(s_sem, 48)
                if m == NPASS - 1:
                    vector.wait_ge(t_sem, 16)  # only pass 4 reads tail strips
                if m >= NBUF:
                    vector.wait_ge(o_sem, 16 * (m - NBUF + 1))
                vector.tensor_copy(
                    bass.AP(
                        ys[m % NBUF],
                        0,
                        [[YF, 128], [ROWLEN, VROWS], [1, ROWLEN]],
                    ),
                    bass.AP(
                        xs,
                        m * NR * C,
                        [[PF, 128], [C, VROWS], [1, ROWLEN]],
                    ),
                ).then_inc(uv_sem, 1)

    return nc


def kernel(x: np.ndarray) -> np.ndarray:
    from concourse.bass_utils import run_bass_kernel_spmd

    x = np.ascontiguousarray(np.asarray(x), dtype=np.float32)
    assert x.shape == (B, T, C), x.shape

    nc = _build_nc()
    in_maps = [{"x": x[i * B_C : (i + 1) * B_C]} for i in range(N_CORES)]
    res = run_bass_kernel_spmd(nc, in_maps, core_ids=list(range(N_CORES)))
    return np.concatenate([r["out"] for r in res.results], axis=0)


# revision 27
# speedup vs baseline: 1.0209x; 1.0209x over previous
"""Overlapping-windows (conv1d-identity unfold) kernel for Trainium2.

out[b*T + t, w*C + c] = x[b, t + w - CTX, c]  (zero-padded in t), i.e. each
output row is a contiguous 494-element window of the zero-padded, flattened
per-batch [T + 2*CTX, C] array starting at t*C.

Strategy:
  - Shard batch dim across 8 cores (8 batches/core).
  - Per core, stage the padded input in SBUF as 128 partitions =
    8 batches x 16 time-chunks; partition p = 16*b + j holds padded rows
    [j*K, j*K + K + 2*CTX) of batch b  (K = 125 rows, 3718 f32/partition).
    Since T*C == NCHUNK*K*C, the per-partition source offset is affine in
    p, so the halo load is a couple of line-rate DMAs over partitions
    1..126 (edge partitions clip at the tensor bounds and load from the
    scalar ring); the cross-batch garbage this leaves in the 18-row halo
    strips at batch boundaries is overwritten by zero-strip DMAs from a
    NEFF-embedded constant, semaphore-ordered after the waves (same-ring
    FIFO is NOT partition-ordered across DMAs — verified empirically).
  - Pipelined unfold passes: DVE + ACT copy-unfold output rows per
    partition into triple-buffered tiles ys[128, 25*494] (per-partition
    strided overlapping reads from xs); outbound DMAs with multi-KB
    contiguous descriptors (both sides contiguous per partition) then
    write at the HBM roofline. Big descriptors amortize the
    per-descriptor DMA-engine overhead that limits a direct
    1976 B-descriptor store to ~250 GB/s. Pass 0 is sub-split (5/6/14
    rows) so the first outbound launches after only 5 rows are unfolded.
"""

import numpy as np

N_CTX = 9
C = 26
W = 2 * N_CTX + 1          # 19
ROWLEN = W * C             # 494
B, T = 64, 2000
N_CORES = 8
B_C = B // N_CORES         # 8 batches per core
NCHUNK = 16                # time-chunks per batch -> 8*16 = 128 partitions
K = T // NCHUNK            # 125 rows per chunk
PF = (K + 2 * N_CTX) * C   # 3718 f32 per partition (chunk + halo)
EDGE = (K + N_CTX) * C     # 3484 f32 (edge partitions, one-sided halo)
HALO = N_CTX * C           # 234 f32

NPASS = 5
NR = K // NPASS            # 25 output rows per partition per pass
VROWS = 11                 # rows unfolded by DVE per pass (ACT does the rest)
NBUF = 3                   # ys staging buffers
YF = NR * ROWLEN           # 12350 f32 per partition per staging buffer
W1A = 256                  # wave-1a columns (covers the head strips)
W1 = (NR + 2 * N_CTX) * C  # 1118 f32: wave-1 columns (rows [0,43) per chunk)
SUB0 = 5                   # rows in the first DVE sub-pass of pass 0


def _build_nc():
    import concourse.bass as bass
    import concourse.mybir as mybir

    nc = bass.Bass(target_bir_lowering=False)
    x = nc.dram_tensor("x", [B_C, T, C], mybir.dt.float32, kind="ExternalInput")
    out = nc.dram_tensor(
        "out", [B_C * T, ROWLEN], mybir.dt.float32, kind="ExternalOutput"
    )

    with (
        nc.sbuf_tensor("xs", [128, PF], mybir.dt.float32) as xs,
        nc.sbuf_tensor("ys0", [128, YF], mybir.dt.float32) as ys0,
        nc.sbuf_tensor("ys1", [128, YF], mybir.dt.float32) as ys1,
        nc.sbuf_tensor("ys2", [128, YF], mybir.dt.float32) as ys2,
        nc.semaphore("in_sem") as in_sem,
        nc.semaphore("s_sem") as s_sem,
        nc.semaphore("t_sem") as t_sem,
        nc.semaphore("uv_sem") as uv_sem,
        nc.semaphore("ua_sem") as ua_sem,
        nc.semaphore("o0_sem") as o0_sem,
        nc.semaphore("o1_sem") as o1_sem,
        nc.semaphore("o2_sem") as o2_sem,
        nc.Block() as block,
    ):
        ys = [ys0, ys1, ys2]
        o_sems = [o0_sem, o1_sem, o2_sem]
        # per-buffer reader counts: ys0 <- segs 0a,0b,0c + pass 3;
        # ys1 <- pass 1 + pass 4; ys2 <- pass 2
        o_final = [16 * 4, 16 * 2, 16 * 1]

        zeros = nc.inline_tensor(
            np.zeros(B_C * HALO, dtype=np.float32), name="zstrip"
        )

        # (dve_rows, act_rows, ys_buf, row_base) per unfold step; pass 0 is
        # a single ys0 tile but its DVE share is sub-split for early start.
        # uv_sem counts DVE copies, ua_sem counts ACT copies.

        # outbound segments: (ys buf, col0, col1, uv_need, ua_need, row0)
        osegs = []
        osegs.append((0, 0, SUB0 * ROWLEN, 1, 0, 0))
        osegs.append((0, SUB0 * ROWLEN, VROWS * ROWLEN, 2, 0, SUB0))
        osegs.append((0, VROWS * ROWLEN, YF, 2, 1, VROWS))
        for m in range(1, NPASS):
            osegs.append((m % NBUF, 0, YF, 2 + m, 1 + m, m * NR))

        @block.sync
        def _(sync):
            # wave 1a: columns [0, W1A) of partitions 1..126; src offset for
            # partition p is p*K*C - HALO (affine because T*C == NCHUNK*K*C)
            sync.dma_start(
                bass.AP(xs, PF, [[PF, 126], [1, W1A]]),
                bass.AP(x, K * C - HALO, [[K * C, 126], [1, W1A]]),
            ).then_inc(in_sem, 16)
            # wave 1b: columns [W1A, W1)
            sync.dma_start(
                bass.AP(xs, PF + W1A, [[PF, 126], [1, W1 - W1A]]),
                bass.AP(x, K * C - HALO + W1A, [[K * C, 126], [1, W1 - W1A]]),
            ).then_inc(in_sem, 16)
            # wave 2: columns [W1, PF)
            sync.dma_start(
                bass.AP(xs, PF + W1, [[PF, 126], [1, PF - W1]]),
                bass.AP(x, K * C - HALO + W1, [[K * C, 126], [1, PF - W1]]),
            ).then_inc(in_sem, 16)

            # ---- outbound: big contiguous descriptors ----
            for buf, c0, c1, uvn, uan, r0 in osegs:
                if uvn:
                    sync.wait_ge(uv_sem, uvn)
                if uan:
                    sync.wait_ge(ua_sem, uan)
                sync.dma_start(
                    bass.AP(
                        out,
                        r0 * ROWLEN,
                        [[K * ROWLEN, 128], [1, c1 - c0]],
                    ),
                    bass.AP(ys[buf], c0, [[YF, 128], [1, c1 - c0]]),
                ).then_inc(o_sems[buf], 16)
            for i, tgt in enumerate(o_final):
                sync.wait_ge(o_sems[i], tgt)

        @block.scalar
        def _(scalar):
            # dummy 1-element copy to preload the ACT identity table during
            # the inbound phase (harmless: ys2 is fully rewritten by its
            # unfold pass before any outbound reads it)
            scalar.copy(
                bass.AP(ys2, 0, [[YF, 1], [1, 1]]),
                bass.AP(xs, 0, [[PF, 1], [1, 1]]),
            )
            # edge partitions: p=0 (batch 0 head, dst offset HALO) and
            # p=127 (batch 7 tail, dst offset 0), each 134 rows
            scalar.dma_start(
                bass.AP(xs, HALO, [[PF, 1], [1, EDGE]]),
                bass.AP(x, 0, [[EDGE, 1], [1, EDGE]]),
            ).then_inc(s_sem, 16)
            scalar.dma_start(
                bass.AP(xs, 127 * PF, [[PF, 1], [1, EDGE]]),
                bass.AP(x, 127 * K * C - HALO, [[EDGE, 1], [1, EDGE]]),
            ).then_inc(s_sem, 16)
            # head zero strips (chunk 0 of each batch, cols [0, HALO)) —
            # must follow wave 1a, which writes cross-batch garbage there
            scalar.wait_ge(in_sem, 16)
            scalar.dma_start(
                bass.AP(xs, 0, [[NCHUNK * PF, B_C], [1, HALO]]),
                bass.AP(zeros, 0, [[HALO, B_C], [1, HALO]]),
            ).then_inc(s_sem, 16)

            # unfold rows [m*NR+VROWS, (m+1)*NR) of each partition
            for m in range(NPASS):
                scalar.wait_ge(in_sem, 32 if m == 0 else 48)
                scalar.wait_ge(s_sem, 48)
                if m == NPASS - 1:
                    scalar.wait_ge(t_sem, 16)  # pass 4 reads tail strips
                if m >= NBUF:
                    # ys[m%NBUF] reuse: all outbound segs of pass m-NBUF done
                    scalar.wait_ge(o_sems[m % NBUF], 48 if m == NBUF else 16)
                scalar.copy(
                    bass.AP(
                        ys[m % NBUF],
                        VROWS * ROWLEN,
                        [[YF, 128], [ROWLEN, NR - VROWS], [1, ROWLEN]],
                    ),
                    bass.AP(
                        xs,
                        (m * NR + VROWS) * C,
                        [[PF, 128], [C, NR - VROWS], [1, ROWLEN]],
                    ),
                ).then_inc(ua_sem, 1)

        @block.gpsimd
        def _(gpsimd):
            # tail zero strips (chunk 15 of each batch, cols [EDGE, PF)):
            # wave 2 writes cross-batch garbage there, so wait for it; only
            # unfold pass 4 reads this region, so there's ample slack.
            gpsimd.wait_ge(in_sem, 48)
            gpsimd.dma_start(
                bass.AP(
                    xs,
                    (NCHUNK - 1) * PF + EDGE,
                    [[NCHUNK * PF, B_C], [1, HALO]],
                ),
                bass.AP(zeros, 0, [[HALO, B_C], [1, HALO]]),
            ).then_inc(t_sem, 16)

        @block.vector
        def _(vector):
            # unfold rows [m*NR, m*NR+VROWS) of each partition; pass 0 is
            # sub-split into [0,SUB0) + [SUB0,VROWS) for an early outbound
            dve_steps = [(0, 0, SUB0), (0, SUB0, VROWS)]
            for m in range(1, NPASS):
                dve_steps.append((m, m * NR, m * NR + VROWS))
            for m, r0, r1 in dve_steps:
                vector.wait_ge(in_sem, 32 if m == 0 else 48)
                vector.wait_ge(s_sem, 48)
                if m >= NBUF:
                    vector.wait_ge(o_sems[m % NBUF], 48 if m == NBUF else 16)
                vector.tensor_copy(
                    bass.AP(
                        ys[m % NBUF],
                        (r0 - m * NR) * ROWLEN,
                        [[YF, 128], [ROWLEN, r1 - r0], [1, ROWLEN]],
                    ),
                    bass.AP(
                        xs,
                        r0 * C,
                        [[PF, 128], [C, r1 - r0], [1, ROWLEN]],
                    ),
                ).then_inc(uv_sem, 1)

    return nc


def kernel(x: np.ndarray) -> np.ndarray:
    from concourse.bass_utils import run_bass_kernel_spmd

    x = np.ascontiguousarray(np.asarray(x), dtype=np.float32)
    assert x.shape == (B, T, C), x.shape

    nc = _build_nc()
    in_maps = [{"x": x[i * B_C : (i + 1) * B_C]} for i in range(N_CORES)]
    res = run_bass_kernel_spmd(nc, in_maps, core_ids=list(range(N_CORES)))
    return np.concatenate([r["out"] for r in res.results], axis=0)


# revision 29
# speedup vs baseline: 1.0616x; 1.0399x over previous
"""Overlapping-windows (conv1d-identity unfold) kernel for Trainium2.

out[b*T + t, w*C + c] = x[b, t + w - CTX, c]  (zero-padded in t), i.e. each
output row is a contiguous 494-element window of the zero-padded, flattened
per-batch [T + 2*CTX, C] array starting at t*C.

Strategy:
  - Shard batch dim across 8 cores (8 batches/core).
  - Per core, stage the padded input in SBUF as 128 partitions =
    8 batches x 16 time-chunks; partition p = 16*b + j holds padded rows
    [j*K, j*K + K + 2*CTX) of batch b  (K = 125 rows, 3718 f32/partition).
    Since T*C == NCHUNK*K*C, the per-partition source offset is affine in
    p, so the halo load is a few line-rate DMAs over partitions 1..126
    (edge partitions clip at the tensor bounds and load from the scalar
    ring); the cross-batch garbage this leaves in the 18-row halo strips
    at batch boundaries is overwritten by zero-strip DMAs from a
    NEFF-embedded constant, semaphore-ordered after the waves (same-ring
    FIFO is NOT partition-ordered across DMAs — verified empirically).
  - Pipelined unfold passes: DVE + ACT copy-unfold output rows per
    partition into triple-buffered tiles ys[128, 25*494] (per-partition
    strided overlapping reads from xs); outbound DMAs with multi-KB
    contiguous descriptors (both sides contiguous per partition) then
    write at the HBM roofline. Big descriptors amortize the
    per-descriptor DMA-engine overhead that limits a direct
    1976 B-descriptor store to ~250 GB/s. Pass 0 is sub-split with the
    strip-independent rows [9,14) first, so the first outbound launches
    before the zero strips are even needed.

Semaphore discipline: a wait threshold is only trusted when reaching it
implies ALL DMAs incrementing that sem so far are complete (per-engine
completion increments from different DMAs are unordered otherwise).
"""

import numpy as np

N_CTX = 9
C = 26
W = 2 * N_CTX + 1          # 19
ROWLEN = W * C             # 494
B, T = 64, 2000
N_CORES = 8
B_C = B // N_CORES         # 8 batches per core
NCHUNK = 16                # time-chunks per batch -> 8*16 = 128 partitions
K = T // NCHUNK            # 125 rows per chunk
PF = (K + 2 * N_CTX) * C   # 3718 f32 per partition (chunk + halo)
EDGE = (K + N_CTX) * C     # 3484 f32 (edge partitions, one-sided halo)
HALO = N_CTX * C           # 234 f32

NPASS = 5
NR = K // NPASS            # 25 output rows per partition per pass
VROWS = 11                 # rows unfolded by DVE per steady pass
NBUF = 3                   # ys staging buffers
YF = NR * ROWLEN           # 12350 f32 per partition per staging buffer
W1A = 256                  # wave-1a columns (covers the head strips)
W1 = (NR + 2 * N_CTX) * C  # 1118 f32: wave-1 columns (rows [0,43) per chunk)


def _build_nc():
    import concourse.bass as bass
    import concourse.mybir as mybir

    nc = bass.Bass(target_bir_lowering=False)
    x = nc.dram_tensor("x", [B_C, T, C], mybir.dt.float32, kind="ExternalInput")
    out = nc.dram_tensor(
        "out", [B_C * T, ROWLEN], mybir.dt.float32, kind="ExternalOutput"
    )

    with (
        nc.sbuf_tensor("xs", [128, PF], mybir.dt.float32) as xs,
        nc.sbuf_tensor("ys0", [128, YF], mybir.dt.float32) as ys0,
        nc.sbuf_tensor("ys1", [128, YF], mybir.dt.float32) as ys1,
        nc.sbuf_tensor("ys2", [128, YF], mybir.dt.float32) as ys2,
        nc.semaphore("in_sem") as in_sem,    # wave1a + wave1b (sync ring)
        nc.semaphore("in2_sem") as in2_sem,  # wave2
        nc.semaphore("e_sem") as e_sem,      # edge partitions p=0, p=127
        nc.semaphore("h_sem") as h_sem,      # head zero strips
        nc.semaphore("t_sem") as t_sem,      # tail zero strips
        nc.semaphore("uv_sem") as uv_sem,    # DVE unfold copies
        nc.semaphore("ua_sem") as ua_sem,    # ACT unfold copies
        nc.semaphore("o0_sem") as o0_sem,    # outbound readers of ys0
        nc.semaphore("o1_sem") as o1_sem,    # outbound readers of ys1
        nc.semaphore("o2_sem") as o2_sem,    # outbound readers of ys2
        nc.Block() as block,
    ):
        ys = [ys0, ys1, ys2]
        o_sems = [o0_sem, o1_sem, o2_sem]

        zeros = nc.inline_tensor(
            np.zeros(B_C * HALO, dtype=np.float32), name="zstrip"
        )

        # unfold steps: DVE does rows [9,14) of pass 0 first (those read
        # cols [HALO, 832) — no zero-strip bytes), then [0,9); ACT does
        # [14,25). Steady passes m>=1: DVE [25m, 25m+VROWS), ACT the rest.
        # outbound segments, in dispatch order:
        #   (buf, row0, nrows, uv_need, ua_need)
        osegs = [
            (0, 9, 5, 1, 0),
            (0, 14, 11, 0, 1),
            (0, 0, 9, 2, 0),
        ]
        for m in range(1, NPASS):
            osegs.append((m % NBUF, m * NR, NR, 2 + m, 1 + m))
        # per-buffer reader counts: ys0 <- 3 segs + pass 3; ys1 <- pass 1 +
        # pass 4; ys2 <- pass 2
        o_final = [16 * 4, 16 * 2, 16 * 1]

        @block.sync
        def _(sync):
            # wave 1a: columns [0, W1A) of partitions 1..126; src offset for
            # partition p is p*K*C - HALO (affine because T*C == NCHUNK*K*C)
            sync.dma_start(
                bass.AP(xs, PF, [[PF, 126], [1, W1A]]),
                bass.AP(x, K * C - HALO, [[K * C, 126], [1, W1A]]),
            ).then_inc(in_sem, 16)
            # wave 1b: columns [W1A, W1)
            sync.dma_start(
                bass.AP(xs, PF + W1A, [[PF, 126], [1, W1 - W1A]]),
                bass.AP(x, K * C - HALO + W1A, [[K * C, 126], [1, W1 - W1A]]),
            ).then_inc(in_sem, 16)
            # wave 2: columns [W1, PF)
            sync.dma_start(
                bass.AP(xs, PF + W1, [[PF, 126], [1, PF - W1]]),
                bass.AP(x, K * C - HALO + W1, [[K * C, 126], [1, PF - W1]]),
            ).then_inc(in2_sem, 16)

            # ---- outbound: big contiguous descriptors ----
            for buf, r0, nr, uvn, uan in osegs:
                if uvn:
                    sync.wait_ge(uv_sem, uvn)
                if uan:
                    sync.wait_ge(ua_sem, uan)
                sync.dma_start(
                    bass.AP(
                        out,
                        r0 * ROWLEN,
                        [[K * ROWLEN, 128], [1, nr * ROWLEN]],
                    ),
                    bass.AP(
                        ys[buf],
                        (r0 - (r0 // NR) * NR) * ROWLEN,
                        [[YF, 128], [1, nr * ROWLEN]],
                    ),
                ).then_inc(o_sems[buf], 16)
            for i, tgt in enumerate(o_final):
                sync.wait_ge(o_sems[i], tgt)

        @block.scalar
        def _(scalar):
            # dummy 1-element copy to preload the ACT identity table during
            # the inbound phase (harmless: ys2 is fully rewritten by its
            # unfold pass before any outbound reads it)
            scalar.copy(
                bass.AP(ys2, 0, [[YF, 1], [1, 1]]),
                bass.AP(xs, 0, [[PF, 1], [1, 1]]),
            )
            # edge partitions: p=0 (batch 0 head, dst offset HALO) and
            # p=127 (batch 7 tail, dst offset 0), each 134 rows
            scalar.dma_start(
                bass.AP(xs, HALO, [[PF, 1], [1, EDGE]]),
                bass.AP(x, 0, [[EDGE, 1], [1, EDGE]]),
            ).then_inc(e_sem, 16)
            scalar.dma_start(
                bass.AP(xs, 127 * PF, [[PF, 1], [1, EDGE]]),
                bass.AP(x, 127 * K * C - HALO, [[EDGE, 1], [1, EDGE]]),
            ).then_inc(e_sem, 16)
            # head zero strips (chunk 0 of each batch, cols [0, HALO)) —
            # must follow wave 1a, which writes cross-batch garbage there
            scalar.wait_ge(in_sem, 16)
            scalar.dma_start(
                bass.AP(xs, 0, [[NCHUNK * PF, B_C], [1, HALO]]),
                bass.AP(zeros, 0, [[HALO, B_C], [1, HALO]]),
            ).then_inc(h_sem, 16)

            # ACT unfold: pass 0 rows [14,25), then [25m+VROWS, (m+1)*NR)
            for m in range(NPASS):
                r0 = 14 if m == 0 else m * NR + VROWS
                r1 = (m + 1) * NR
                scalar.wait_ge(in_sem, 32)
                scalar.wait_ge(e_sem, 32)
                if m >= 1:
                    scalar.wait_ge(in2_sem, 16)
                if m == NPASS - 1:
                    scalar.wait_ge(t_sem, 16)  # pass 4 reads tail strips
                if m >= NBUF:
                    scalar.wait_ge(o_sems[m % NBUF], 48 if m == NBUF else 16)
                scalar.copy(
                    bass.AP(
                        ys[m % NBUF],
                        (r0 - m * NR) * ROWLEN,
                        [[YF, 128], [ROWLEN, r1 - r0], [1, ROWLEN]],
                    ),
                    bass.AP(
                        xs,
                        r0 * C,
                        [[PF, 128], [C, r1 - r0], [1, ROWLEN]],
                    ),
                ).then_inc(ua_sem, 1)

        @block.gpsimd
        def _(gpsimd):
            # tail zero strips (chunk 15 of each batch, cols [EDGE, PF)):
            # wave 2 writes cross-batch garbage there, so wait for it; only
            # unfold pass 4 reads this region, so there's ample slack.
            gpsimd.wait_ge(in2_sem, 16)
            gpsimd.dma_start(
                bass.AP(
                    xs,
                    (NCHUNK - 1) * PF + EDGE,
                    [[NCHUNK * PF, B_C], [1, HALO]],
                ),
                bass.AP(zeros, 0, [[HALO, B_C], [1, HALO]]),
            ).then_inc(t_sem, 16)

        @block.vector
        def _(vector):
            # DVE unfold: pass 0 sub-split into rows [9,14) (strip-free,
            # launches the first outbound) then [0,9) (needs head strips);
            # steady passes m>=1 do rows [25m, 25m+VROWS)
            dve_steps = [(0, 9, 14, False), (0, 0, 9, True)]
            for m in range(1, NPASS):
                dve_steps.append((m, m * NR, m * NR + VROWS, False))
            for m, r0, r1, needs_strips in dve_steps:
                vector.wait_ge(in_sem, 32)
                vector.wait_ge(e_sem, 32)
                if needs_strips:
                    vector.wait_ge(h_sem, 16)
                if m >= 1:
                    vector.wait_ge(in2_sem, 16)
                if m >= NBUF:
                    vector.wait_ge(o_sems[m % NBUF], 48 if m == NBUF else 16)
                vector.tensor_copy(
                    bass.AP(
                        ys[m % NBUF],
                        (r0 - m * NR) * ROWLEN,
                        [[YF, 128], [ROWLEN, r1 - r0], [1, ROWLEN]],
                    ),
                    bass.AP(
                        xs,
                        r0 * C,
                        [[PF, 128], [C, r1 - r0], [1, ROWLEN]],
                    ),
                ).then_inc(uv_sem, 1)

    return nc


def kernel(x: np.ndarray) -> np.ndarray:
    from concourse.bass_utils import run_bass_kernel_spmd

    x = np.ascontiguousarray(np.asarray(x), dtype=np.float32)
    assert x.shape == (B, T, C), x.shape

    nc = _build_nc()
    in_maps = [{"x": x[i * B_C : (i + 1) * B_C]} for i in range(N_CORES)]
    res = run_bass_kernel_spmd(nc, in_maps, core_ids=list(range(N_CORES)))
    return np.concatenate([r["out"] for r in res.results], axis=0)


# revision 30
# speedup vs baseline: 1.0775x; 1.0150x over previous
"""Overlapping-windows (conv1d-identity unfold) kernel for Trainium2.

out[b*T + t, w*C + c] = x[b, t + w - CTX, c]  (zero-padded in t), i.e. each
output row is a contiguous 494-element window of the zero-padded, flattened
per-batch [T + 2*CTX, C] array starting at t*C.

Strategy:
  - Shard batch dim across 8 cores (8 batches/core).
  - Per core, stage the padded input in SBUF as 128 partitions =
    8 batches x 16 time-chunks; partition p = 16*b + j holds padded rows
    [j*K, j*K + K + 2*CTX) of batch b  (K = 125 rows, 3718 f32/partition).
    Since T*C == NCHUNK*K*C, the per-partition source offset is affine in
    p, so the halo load is a few line-rate DMAs over partitions 1..126
    (edge partitions clip at the tensor bounds and load from the scalar
    ring); the cross-batch garbage this leaves in the 18-row halo strips
    at batch boundaries is overwritten by zero-strip DMAs from a
    NEFF-embedded constant, semaphore-ordered after the waves (same-ring
    FIFO is NOT partition-ordered across DMAs — verified empirically).
  - Pipelined unfold passes: DVE + ACT copy-unfold output rows per
    partition into triple-buffered tiles ys[128, 25*494] (per-partition
    strided overlapping reads from xs); outbound DMAs with multi-KB
    contiguous descriptors (both sides contiguous per partition) then
    write at the HBM roofline. Big descriptors amortize the
    per-descriptor DMA-engine overhead that limits a direct
    1976 B-descriptor store to ~250 GB/s. Pass 0 is sub-split with the
    strip-independent rows [9,14) first, so the first outbound launches
    before the zero strips are even needed.

Semaphore discipline: a wait threshold is only trusted when reaching it
implies ALL DMAs incrementing that sem so far are complete (per-engine
completion increments from different DMAs are unordered otherwise).
"""

import numpy as np

N_CTX = 9
C = 26
W = 2 * N_CTX + 1          # 19
ROWLEN = W * C             # 494
B, T = 64, 2000
N_CORES = 8
B_C = B // N_CORES         # 8 batches per core
NCHUNK = 16                # time-chunks per batch -> 8*16 = 128 partitions
K = T // NCHUNK            # 125 rows per chunk
PF = (K + 2 * N_CTX) * C   # 3718 f32 per partition (chunk + halo)
EDGE = (K + N_CTX) * C     # 3484 f32 (edge partitions, one-sided halo)
HALO = N_CTX * C           # 234 f32

NPASS = 5
NR = K // NPASS            # 25 output rows per partition per pass
VROWS = 11                 # rows unfolded by DVE per steady pass
NBUF = 3                   # ys staging buffers
YF = NR * ROWLEN           # 12350 f32 per partition per staging buffer
W1A = 832                  # wave-1a cols: strips + all DVE sub-pass 0a reads
W1 = (NR + 2 * N_CTX) * C  # 1118 f32: wave-1 columns (rows [0,43) per chunk)


def _build_nc():
    import concourse.bass as bass
    import concourse.mybir as mybir

    nc = bass.Bass(target_bir_lowering=False)
    x = nc.dram_tensor("x", [B_C, T, C], mybir.dt.float32, kind="ExternalInput")
    out = nc.dram_tensor(
        "out", [B_C * T, ROWLEN], mybir.dt.float32, kind="ExternalOutput"
    )

    with (
        nc.sbuf_tensor("xs", [128, PF], mybir.dt.float32) as xs,
        nc.sbuf_tensor("ys0", [128, YF], mybir.dt.float32) as ys0,
        nc.sbuf_tensor("ys1", [128, YF], mybir.dt.float32) as ys1,
        nc.sbuf_tensor("ys2", [128, YF], mybir.dt.float32) as ys2,
        nc.semaphore("in_sem") as in_sem,    # wave1a + wave1b (sync ring)
        nc.semaphore("in2_sem") as in2_sem,  # wave2
        nc.semaphore("e_sem") as e_sem,      # edge partitions p=0, p=127
        nc.semaphore("h_sem") as h_sem,      # head zero strips
        nc.semaphore("t_sem") as t_sem,      # tail zero strips
        nc.semaphore("uv_sem") as uv_sem,    # DVE unfold copies
        nc.semaphore("ua_sem") as ua_sem,    # ACT unfold copies
        nc.semaphore("o0_sem") as o0_sem,    # outbound readers of ys0
        nc.semaphore("o1_sem") as o1_sem,    # outbound readers of ys1
        nc.semaphore("o2_sem") as o2_sem,    # outbound readers of ys2
        nc.Block() as block,
    ):
        ys = [ys0, ys1, ys2]
        o_sems = [o0_sem, o1_sem, o2_sem]

        zeros = nc.inline_tensor(
            np.zeros(B_C * HALO, dtype=np.float32), name="zstrip"
        )

        # unfold steps: DVE does rows [9,14) of pass 0 first (those read
        # cols [HALO, 832) — no zero-strip bytes), then [0,9); ACT does
        # [14,25). Steady passes m>=1: DVE [25m, 25m+VROWS), ACT the rest.
        # outbound segments, in dispatch order:
        #   (buf, row0, nrows, uv_need, ua_need)
        osegs = [
            (0, 9, 5, 1, 0),
            (0, 14, 11, 0, 1),
            (0, 0, 9, 2, 0),
        ]
        for m in range(1, NPASS):
            osegs.append((m % NBUF, m * NR, NR, 2 + m, 1 + m))
        # per-buffer reader counts: ys0 <- 3 segs + pass 3; ys1 <- pass 1 +
        # pass 4; ys2 <- pass 2
        o_final = [16 * 4, 16 * 2, 16 * 1]

        @block.sync
        def _(sync):
            # wave 1a: columns [0, W1A) of partitions 1..126; src offset for
            # partition p is p*K*C - HALO (affine because T*C == NCHUNK*K*C)
            sync.dma_start(
                bass.AP(xs, PF, [[PF, 126], [1, W1A]]),
                bass.AP(x, K * C - HALO, [[K * C, 126], [1, W1A]]),
            ).then_inc(in_sem, 16)
            # wave 1b: columns [W1A, W1)
            sync.dma_start(
                bass.AP(xs, PF + W1A, [[PF, 126], [1, W1 - W1A]]),
                bass.AP(x, K * C - HALO + W1A, [[K * C, 126], [1, W1 - W1A]]),
            ).then_inc(in_sem, 16)
            # wave 2: columns [W1, PF)
            sync.dma_start(
                bass.AP(xs, PF + W1, [[PF, 126], [1, PF - W1]]),
                bass.AP(x, K * C - HALO + W1, [[K * C, 126], [1, PF - W1]]),
            ).then_inc(in2_sem, 16)

            # ---- outbound: big contiguous descriptors ----
            for buf, r0, nr, uvn, uan in osegs:
                if uvn:
                    sync.wait_ge(uv_sem, uvn)
                if uan:
                    sync.wait_ge(ua_sem, uan)
                sync.dma_start(
                    bass.AP(
                        out,
                        r0 * ROWLEN,
                        [[K * ROWLEN, 128], [1, nr * ROWLEN]],
                    ),
                    bass.AP(
                        ys[buf],
                        (r0 - (r0 // NR) * NR) * ROWLEN,
                        [[YF, 128], [1, nr * ROWLEN]],
                    ),
                ).then_inc(o_sems[buf], 16)
            for i, tgt in enumerate(o_final):
                sync.wait_ge(o_sems[i], tgt)

        @block.scalar
        def _(scalar):
            # dummy 1-element copy to preload the ACT identity table during
            # the inbound phase (harmless: ys2 is fully rewritten by its
            # unfold pass before any outbound reads it)
            scalar.copy(
                bass.AP(ys2, 0, [[YF, 1], [1, 1]]),
                bass.AP(xs, 0, [[PF, 1], [1, 1]]),
            )
            # edge partitions: p=0 (batch 0 head, dst offset HALO) and
            # p=127 (batch 7 tail, dst offset 0), each 134 rows
            scalar.dma_start(
                bass.AP(xs, HALO, [[PF, 1], [1, EDGE]]),
                bass.AP(x, 0, [[EDGE, 1], [1, EDGE]]),
            ).then_inc(e_sem, 16)
            scalar.dma_start(
                bass.AP(xs, 127 * PF, [[PF, 1], [1, EDGE]]),
                bass.AP(x, 127 * K * C - HALO, [[EDGE, 1], [1, EDGE]]),
            ).then_inc(e_sem, 16)
            # head zero strips (chunk 0 of each batch, cols [0, HALO)) —
            # must follow wave 1a, which writes cross-batch garbage there
            scalar.wait_ge(in_sem, 16)
            scalar.dma_start(
                bass.AP(xs, 0, [[NCHUNK * PF, B_C], [1, HALO]]),
                bass.AP(zeros, 0, [[HALO, B_C], [1, HALO]]),
            ).then_inc(h_sem, 16)

            # ACT unfold: pass 0 rows [14,25), then [25m+VROWS, (m+1)*NR)
            for m in range(NPASS):
                r0 = 14 if m == 0 else m * NR + VROWS
                r1 = (m + 1) * NR
                scalar.wait_ge(in_sem, 32)
                scalar.wait_ge(e_sem, 32)
                if m >= 1:
                    scalar.wait_ge(in2_sem, 16)
                if m == NPASS - 1:
                    scalar.wait_ge(t_sem, 16)  # pass 4 reads tail strips
                if m >= NBUF:
                    scalar.wait_ge(o_sems[m % NBUF], 48 if m == NBUF else 16)
                scalar.copy(
                    bass.AP(
                        ys[m % NBUF],
                        (r0 - m * NR) * ROWLEN,
                        [[YF, 128], [ROWLEN, r1 - r0], [1, ROWLEN]],
                    ),
                    bass.AP(
                        xs,
                        r0 * C,
                        [[PF, 128], [C, r1 - r0], [1, ROWLEN]],
                    ),
                ).then_inc(ua_sem, 1)

        @block.gpsimd
        def _(gpsimd):
            # tail zero strips (chunk 15 of each batch, cols [EDGE, PF)):
            # wave 2 writes cross-batch garbage there, so wait for it; only
            # unfold pass 4 reads this region, so there's ample slack.
            gpsimd.wait_ge(in2_sem, 16)
            gpsimd.dma_start(
                bass.AP(
                    xs,
                    (NCHUNK - 1) * PF + EDGE,
                    [[NCHUNK * PF, B_C], [1, HALO]],
                ),
                bass.AP(zeros, 0, [[HALO, B_C], [1, HALO]]),
            ).then_inc(t_sem, 16)

        @block.vector
        def _(vector):
            # DVE unfold: pass 0 sub-split into rows [9,14) (strip-free,
            # launches the first outbound) then [0,9) (needs head strips);
            # steady passes m>=1 do rows [25m, 25m+VROWS)
            dve_steps = [(0, 9, 14, False, 16), (0, 0, 9, True, 16)]
            for m in range(1, NPASS):
                dve_steps.append((m, m * NR, m * NR + VROWS, False, 32))
            for m, r0, r1, needs_strips, in_need in dve_steps:
                vector.wait_ge(in_sem, in_need)
                vector.wait_ge(e_sem, 32)
                if needs_strips:
                    vector.wait_ge(h_sem, 16)
                if m >= 1:
                    vector.wait_ge(in2_sem, 16)
                if m >= NBUF:
                    vector.wait_ge(o_sems[m % NBUF], 48 if m == NBUF else 16)
                vector.tensor_copy(
                    bass.AP(
                        ys[m % NBUF],
                        (r0 - m * NR) * ROWLEN,
                        [[YF, 128], [ROWLEN, r1 - r0], [1, ROWLEN]],
                    ),
                    bass.AP(
                        xs,
                        r0 * C,
                        [[PF, 128], [C, r1 - r0], [1, ROWLEN]],
                    ),
                ).then_inc(uv_sem, 1)

    return nc


def kernel(x: np.ndarray) -> np.ndarray:
    from concourse.bass_utils import run_bass_kernel_spmd

    x = np.ascontiguousarray(np.asarray(x), dtype=np.float32)
    assert x.shape == (B, T, C), x.shape

    nc = _build_nc()
    in_maps = [{"x": x[i * B_C : (i + 1) * B_C]} for i in range(N_CORES)]
    res = run_bass_kernel_spmd(nc, in_maps, core_ids=list(range(N_CORES)))
    return np.concatenate([r["out"] for r in res.results], axis=0)
